# revision 34
# baseline (speedup 1.0000x reference)
"""Trainium2 Bass kernel for the DEER-MLP spiking network.

Network: x(4,32,196,384) -> FC1(384->1536) -> BatchNorm -> LIF(T=4) ->
FC2(1536->384) -> BatchNorm -> LIF -> spikes(4,32,196,384).

Math note: the reference solves the LIF recurrence with 10 DEER Newton
iterations over T=4 steps. Newton on a length-T triangular system is exact
after T iterations, so the converged result equals the plain sequential
recurrence; we compute that directly (4 elementwise steps).

Distribution: data-parallel over the flattened B*N batch across 8 cores
(784 lanes/core). BatchNorm statistics are the only cross-core coupling,
handled with small AllReduces.

Precision: both matmuls run as multi-pass fp16 with operands split into
hi/lo fp16 limbs (split on host; the PE honors fp16 subnormals). fp16
products accumulate exactly into fp32 PSUM, so FC1 = x_hi@w_hi + x_lo@w_hi
+ x_hi@w_lo reproduces fp32 to ~2^-22, and FC2's spikes are exactly 0/1 in
fp16 so two passes (w_hi + w_lo) are ~2^-22 as well.

Schedule (v2, latency-hiding restructure):
  FC1 runs a-tile-major in channel groups (4,4,3,1). After each group's
  rows finish, its BN1 partial stats AllReduce launches on the gpsimd
  queue while the PE continues FC1 on later groups; BN1 affine + LIF1 for
  the group run on the Vector engine (also overlapped with FC1). The BN
  rsqrt is computed on DVE with a magic-constant Newton iteration so the
  Scalar queue (PSUM evacuation) never blocks on a collective.
  FC2 starts the moment FC1 ends: its first six PSUM tiles accumulate
  k-tiles 0..10 first and defer k=11 (the last group's spikes), hiding
  the final AllReduce+LIF1 latency behind real matmul work.
  BN2 stats AllReduce + coeffs follow the same pattern; LIF2 is split
  across Vector (ct 0,1) and GpSimd (ct 2); the output transpose runs on
  the idle PE (128x128 fp16 transposes into PSUM) instead of serialized
  DMA XBAR transposes, with the fp32 upcast fused into the Scalar-engine
  PSUM evacuation.

Host-side prep in kernel(): shard x over B, pre-transpose to [C, R] and
split into fp16 limbs; pre-transpose W1/W2 and split into fp16 limbs.
"""

import numpy as np

import concourse.bass as bass
import concourse.mybir as mybir
import concourse.tile as tile
from concourse import bacc
from concourse.bass_utils import run_bass_kernel_spmd

F32 = mybir.dt.float32
F16 = mybir.dt.float16
I32 = mybir.dt.int32
AF = mybir.ActivationFunctionType
OP = mybir.AluOpType
AX = mybir.AxisListType

T, B, NN, C, H = 4, 32, 196, 384, 1536
NCORES = 8
BLOC = B // NCORES            # 4 batches per core
MLOC = BLOC * NN              # 784 lanes per core
R = T * MLOC                  # 3136 flattened (t, m) rows per core
NTOT = T * B * NN             # 25088 batchnorm samples per channel
KC = C // 128                 # 3 c-tiles
KH = H // 128                 # 12 h-tiles
EPS = 1e-5
P = 128

GROUPS = [[0, 1, 2, 3], [4, 5, 6, 7], [8, 9, 10], [11]]
A_LAST = 11                   # deferred FC2 k-tile (last stats group)

# FC1 row halves per a-tile. a<11 uses (1536, 1600) to minimize matmul
# count; a=11 uses (1568, 1568) so each half is exactly two t-planes and
# can stay SBUF-resident for the critical-path LIF.
HALVES_STD = [(0, 1536), (1536, 1600)]
HALVES_A11 = [(0, 1568), (1568, 1568)]

# FC2 column chunks of the flattened r axis (one PSUM bank each).
B_CHUNKS = [(i * 512, 512) for i in range(6)] + [(3072, 64)]

MCH = 392                     # LIF1 m-chunk width
MAGIC = 0x5F3759DF


def _subs(cols):
    out, o = [], 0
    while o < cols:
        s = min(512, cols - o)
        out.append((o, s))
        o += s
    return out


def _rsqrt_newton(nc, pool, u, k15, n, tag):
    """out = 1/sqrt(u) on DVE only: magic-constant init + 3 Newton steps.

    u: [P, n] f32 (positive). Returns a [P, n] f32 tile. fp32-rounding
    accurate (~1e-7 rel) after 3 iterations.
    """
    y0i = pool.tile([P, n], I32, tag=f"{tag}_yi", name=f"{tag}_yi")
    nc.vector.tensor_scalar(y0i[:], u.bitcast(I32), 1, None,
                            OP.logical_shift_right)
    # MAGIC - t  ==  t * (-1) + MAGIC  (int32 arithmetic)
    nc.vector.tensor_scalar(y0i[:], y0i[:], -1, MAGIC, OP.mult, OP.add)
    y = y0i[:].bitcast(F32)
    t2 = pool.tile([P, n], F32, tag=f"{tag}_t2", name=f"{tag}_t2")
    for _ in range(3):
        nc.vector.tensor_tensor(t2[:], y, y, OP.mult)
        nc.vector.tensor_tensor(t2[:], t2[:], u, OP.mult)
        nc.vector.scalar_tensor_tensor(t2[:], t2[:], -0.5, k15, OP.mult,
                                       OP.add)
        nc.vector.tensor_tensor(y, y, t2[:], OP.mult)
    return y0i


def _bn_coeffs(nc, pool, stg, gt, bet, dsc, dsh, k15, gs, tag):
    """From allreduced [P, 2*gs] (sum || sumsq) fill dsc/dsh slices with
    the fused affine  drive = y*dsc + dsh
                   == 0.5 * ((y - mean) * rsqrt(var+eps) * g + be).
    All on the Vector engine (no Scalar-queue involvement)."""
    mean = pool.tile([P, gs], F32, tag=f"{tag}_mean", name=f"{tag}_mean")
    nc.vector.tensor_scalar(mean[:], stg[:, 0:gs], 1.0 / NTOT, None, OP.mult)
    u = pool.tile([P, gs], F32, tag=f"{tag}_u", name=f"{tag}_u")
    nc.vector.tensor_scalar(u[:], stg[:, gs:2 * gs], 1.0 / NTOT, None,
                            OP.mult)
    msq = pool.tile([P, gs], F32, tag=f"{tag}_msq", name=f"{tag}_msq")
    nc.vector.tensor_tensor(msq[:], mean[:], mean[:], OP.mult)
    nc.vector.tensor_tensor(u[:], u[:], msq[:], OP.subtract)
    nc.vector.tensor_scalar(u[:], u[:], EPS, None, OP.add)
    rstd = _rsqrt_newton(nc, pool, u[:], k15[:, 0:gs], gs, tag)
    nc.vector.tensor_tensor(dsc, rstd[:].bitcast(F32), gt, OP.mult)
    nc.vector.tensor_scalar(dsc, dsc, 0.5, None, OP.mult)
    nc.vector.tensor_tensor(msq[:], mean[:], dsc, OP.mult)
    nc.vector.scalar_tensor_tensor(dsh, bet, 0.5, msq[:], OP.mult,
                                   OP.subtract)


def _build():
    nc = bacc.Bacc("TRN2", target_bir_lowering=False, debug=False,
                   num_devices=NCORES)

    xh_d = nc.dram_tensor("xthi", [KC, P, R], F16, kind="ExternalInput")
    xl_d = nc.dram_tensor("xtlo", [KC, P, R], F16, kind="ExternalInput")
    w1h_d = nc.dram_tensor("w1thi", [KC, P, H], F16, kind="ExternalInput")
    w1l_d = nc.dram_tensor("w1tlo", [KC, P, H], F16, kind="ExternalInput")
    w2h_d = nc.dram_tensor("w2thi", [KH, P, C], F16, kind="ExternalInput")
    w2l_d = nc.dram_tensor("w2tlo", [KH, P, C], F16, kind="ExternalInput")
    b1_d = nc.dram_tensor("b1", [H], F32, kind="ExternalInput")
    g1_d = nc.dram_tensor("g1", [H], F32, kind="ExternalInput")
    be1_d = nc.dram_tensor("be1", [H], F32, kind="ExternalInput")
    b2_d = nc.dram_tensor("b2", [C], F32, kind="ExternalInput")
    g2_d = nc.dram_tensor("g2", [C], F32, kind="ExternalInput")
    be2_d = nc.dram_tensor("be2", [C], F32, kind="ExternalInput")
    id_d = nc.dram_tensor("ident", [P, P], F16, kind="ExternalInput")
    out_d = nc.dram_tensor("out", [R, C], F32, kind="ExternalOutput")

    groups = [list(range(NCORES))]

    with tile.TileContext(nc) as tc:
        with (
            tc.tile_pool(name="const", bufs=1) as const,
            tc.tile_pool(name="dram", bufs=1, space="DRAM") as dram,
        ):
            # ---- small per-channel constants (gpsimd queue) ----------
            def colvec(dst_k, src):
                t_ = const.tile([P, dst_k], F32, name=f"cv_{src.name}",
                                tag=f"cv_{src.name}")
                nc.gpsimd.dma_start(
                    t_[:], src.ap().rearrange("(a p) -> p a", p=P)
                )
                return t_

            identt = const.tile([P, P], F16)
            nc.gpsimd.dma_start(identt[:], id_d.ap())
            b1t, g1t, be1t = (colvec(KH, d) for d in (b1_d, g1_d, be1_d))
            b2t, g2t, be2t = (colvec(KC, d) for d in (b2_d, g2_d, be2_d))
            k15 = const.tile([P, 8], F32)
            nc.vector.memset(k15[:], 1.5)

            # weights for FC2 (needed only at ~FC1-end, but allocated in
            # the outer pool so the early DMA has a destination).
            w2h = const.tile([P, KH, C], F16)
            w2l = const.tile([P, KH, C], F16)

            # spike tiles (fp16, flat r = t*784+m), alive into FC2
            s1 = [const.tile([P, R], F16, name=f"s1_{a}", tag=f"s1_{a}")
                  for a in range(KH)]

            # stats + affine coefficient tiles
            asum1 = const.tile([P, KH, 2], F32)
            asq1 = const.tile([P, KH, 2], F32)
            dsc1 = const.tile([P, KH], F32)
            dsh1 = const.tile([P, KH], F32)
            asum2 = const.tile([P, KC, 7], F32)
            asq2 = const.tile([P, KC, 7], F32)
            dsc2 = const.tile([P, KC], F32)
            dsh2 = const.tile([P, KC], F32)

            y1s = dram.tile([KH - 1, P, R], F32)
            st_in = [dram.tile([P, 2 * len(g)], F32, name=f"st_in{gi}")
                     for gi, g in enumerate(GROUPS)]
            st_out = [dram.tile([P, 2 * len(g)], F32, name=f"st_out{gi}")
                      for gi, g in enumerate(GROUPS)]
            st2_in = dram.tile([P, 2 * KC], F32)
            st2_out = dram.tile([P, 2 * KC], F32)

            # =========== phase A: FC1 + BN1 stats + LIF1 ==============
            with (
                tc.tile_pool(name="pax", bufs=1) as pax,
                tc.tile_pool(name="pa", bufs=2) as pa,
                tc.tile_pool(name="plif", bufs=2) as plif,
                tc.tile_pool(name="pvh", bufs=4) as pvh,
                tc.tile_pool(name="pco", bufs=2) as pco,
                tc.tile_pool(name="ps_mm", bufs=2, space="PSUM") as ps_mm,
            ):
                # input loads: big x/w tensors split across two queues so
                # the first matmul can start after ~8us.
                w1h = pax.tile([P, KC, H], F16)
                nc.sync.dma_start(w1h[:], w1h_d.ap().rearrange("k p h -> p k h"))
                xh = pax.tile([P, KC, R], F16)
                xh_src = xh_d.ap().rearrange("k p r -> p k r")
                nc.sync.dma_start(xh[:, :, 0:1536], xh_src[:, :, 0:1536])
                nc.sync.dma_start(xh[:, :, 1536:R], xh_src[:, :, 1536:R])
                w1l = pax.tile([P, KC, H], F16)
                nc.gpsimd.dma_start(w1l[:], w1l_d.ap().rearrange("k p h -> p k h"))
                xl = pax.tile([P, KC, R], F16)
                xl_src = xl_d.ap().rearrange("k p r -> p k r")
                nc.gpsimd.dma_start(xl[:, :, 0:1536], xl_src[:, :, 0:1536])
                nc.gpsimd.dma_start(xl[:, :, 1536:R], xl_src[:, :, 1536:R])
                nc.gpsimd.dma_start(w2h[:], w2h_d.ap().rearrange("k p c -> p k c"))
                nc.gpsimd.dma_start(w2l[:], w2l_d.ap().rearrange("k p c -> p k c"))

                y1keep = [pax.tile([P, 1568], F32, name=f"y1keep{h}")
                          for h in range(2)]

                def lif(drive_t, s_t, mlen, tag):
                    """Sequential LIF over T steps on the Vector engine.
                    drive_t / s_t: lists of 4 APs [P, mlen] (drive already
                    affine; s_t are the fp16 spike output slices)."""
                    v = pvh.tile([P, MCH], F32, tag="lv", name=f"{tag}_v")
                    d0 = drive_t[0]
                    nc.vector.scalar_tensor_tensor(v[:, :mlen], d0, 1.0, d0,
                                                   OP.is_lt, OP.mult)
                    nc.vector.tensor_scalar(s_t[0], d0, 1.0, None, OP.is_ge)
                    for t in range(1, T):
                        hh = pvh.tile([P, MCH], F32, tag="lh",
                                      name=f"{tag}_h{t}")
                        nc.vector.scalar_tensor_tensor(hh[:, :mlen], v[:, :mlen],
                                                       0.5, drive_t[t],
                                                       OP.mult, OP.add)
                        if t < T - 1:
                            v = pvh.tile([P, MCH], F32, tag="lv",
                                         name=f"{tag}_v{t}")
                            nc.vector.scalar_tensor_tensor(
                                v[:, :mlen], hh[:, :mlen], 1.0, hh[:, :mlen],
                                OP.is_lt, OP.mult)
                        nc.vector.tensor_scalar(s_t[t], hh[:, :mlen], 1.0,
                                                None, OP.is_ge)

                def emit_ar(gi, group):
                    """Stats pair-sum + AllReduce trigger (gpsimd queue).
                    The result fetch is emitted later (after the NEXT
                    group's trigger) so it cannot head-of-line block that
                    trigger behind this collective's completion."""
                    gs = len(group)
                    a0 = group[0]
                    stg = pco.tile([P, 2 * gs], F32, tag="stg",
                                   name=f"stg{gi}")
                    nc.gpsimd.tensor_tensor(stg[:, 0:gs],
                                            asum1[:, a0:a0 + gs, 0],
                                            asum1[:, a0:a0 + gs, 1], OP.add)
                    nc.gpsimd.tensor_tensor(stg[:, gs:2 * gs],
                                            asq1[:, a0:a0 + gs, 0],
                                            asq1[:, a0:a0 + gs, 1], OP.add)
                    nc.gpsimd.dma_start(st_in[gi][:], stg[:])
                    nc.gpsimd.collective_compute(
                        "AllReduce", OP.add, replica_groups=groups,
                        ins=[st_in[gi].opt()], outs=[st_out[gi].opt()],
                    )

                def emit_lif_group(gi, group):
                    """Fetch allreduced stats (gpsimd), coeffs + BN1
                    affine + LIF on Vector."""
                    gs = len(group)
                    a0 = group[0]
                    stgo = pco.tile([P, 2 * gs], F32, tag="stgo",
                                    name=f"stgo{gi}")
                    nc.gpsimd.dma_start(stgo[:], st_out[gi][:])
                    _bn_coeffs(nc, pco, stgo, g1t[:, a0:a0 + gs],
                               be1t[:, a0:a0 + gs], dsc1[:, a0:a0 + gs],
                               dsh1[:, a0:a0 + gs], k15, gs, f"bc1_{gi}")
                    for a in group:
                        s1v = s1[a].rearrange("p (t m) -> p t m", t=T)
                        if a == A_LAST:
                            for h2 in range(2):
                                nc.vector.tensor_scalar(
                                    y1keep[h2][:], y1keep[h2][:],
                                    dsc1[:, a:a + 1], dsh1[:, a:a + 1],
                                    OP.mult, OP.add)
                            full_t = [y1keep[0][:, 0:MLOC],
                                      y1keep[0][:, MLOC:2 * MLOC],
                                      y1keep[1][:, 0:MLOC],
                                      y1keep[1][:, MLOC:2 * MLOC]]
                            for m0 in range(0, MLOC, MCH):
                                lif([d[:, m0:m0 + MCH] for d in full_t],
                                    [s1v[:, t, m0:m0 + MCH]
                                     for t in range(T)],
                                    MCH, f"l{a}_{m0}")
                        else:
                            for m0 in range(0, MLOC, MCH):
                                yt = plif.tile([P, T, MCH], F32, tag="yt",
                                               name=f"yt{a}_{m0}")
                                src = y1s[a].rearrange("p (t m) -> p t m",
                                                       t=T)
                                nc.sync.dma_start(
                                    yt[:], src[:, :, m0:m0 + MCH])
                                nc.vector.tensor_scalar(
                                    yt[:], yt[:], dsc1[:, a:a + 1],
                                    dsh1[:, a:a + 1], OP.mult, OP.add)
                                lif([yt[:, t, :] for t in range(T)],
                                    [s1v[:, t, m0:m0 + MCH]
                                     for t in range(T)],
                                    MCH, f"l{a}_{m0}")

                for gi, group in enumerate(GROUPS):
                    for a in group:
                        halves = HALVES_A11 if a == A_LAST else HALVES_STD
                        for hi, (base, cols) in enumerate(halves):
                            ps = ps_mm.tile([P, 2048], F32, tag="mmA")
                            li = 0
                            for wt, xt in ((w1h, xh), (w1l, xh), (w1h, xl)):
                                for k in range(KC):
                                    for s0, sl in _subs(cols):
                                        nc.tensor.matmul(
                                            ps[:, s0:s0 + sl],
                                            wt[:, k, a * P:(a + 1) * P],
                                            xt[:, k, base + s0:base + s0 + sl],
                                            start=(li == 0),
                                            stop=(li == 8),
                                        )
                                    li += 1
                            if a == A_LAST:
                                ydst = y1keep[hi][:, :cols]
                            else:
                                y1sb = pa.tile([P, 1600], F32, tag="y1sb")
                                ydst = y1sb[:, :cols]
                            nc.scalar.activation(
                                ydst, ps[:, :cols], AF.Identity,
                                bias=b1t[:, a:a + 1], scale=1.0,
                                accum_out=asum1[:, a, hi:hi + 1],
                            )
                            sqs = pa.tile([P, 1600], F16, tag="sqs")
                            nc.scalar.activation(
                                sqs[:, :cols], ps[:, :cols], AF.Square,
                                bias=b1t[:, a:a + 1], scale=1.0,
                                accum_out=asq1[:, a, hi:hi + 1],
                            )
                            if a != A_LAST:
                                nc.sync.dma_start(
                                    y1s[a][:, base:base + cols], ydst)

                    # AllReduce scheduling: triggers must never queue
                    # behind a result fetch (a fetch completes only when
                    # its collective does). Groups 2/3 are swapped so the
                    # tiny g3 collective (a11 — FC2's deferred k-tile)
                    # exchanges on the CC engine before g2's.
                    if gi <= 1:
                        emit_ar(gi, group)
                    elif gi == 2:
                        emit_lif_group(0, GROUPS[0])
                    else:
                        emit_ar(3, GROUPS[3])
                        emit_ar(2, GROUPS[2])
                        emit_lif_group(1, GROUPS[1])
                        emit_lif_group(3, GROUPS[3])
                        emit_lif_group(2, GROUPS[2])

            # =========== phase B: FC2 + BN2 stats =====================
            with (
                tc.tile_pool(name="pb", bufs=1) as pb,
                tc.tile_pool(name="pbs", bufs=2) as pbs,
                tc.tile_pool(name="pco2", bufs=1) as pco2,
            ):
                y2r = [pb.tile([P, R], F32, name=f"y2r{ct}")
                       for ct in range(KC)]
                s2t = [[pb.tile([P, 896], F16, name=f"s2t{ct}_{t}")
                        for t in range(T)] for ct in range(KC)]
                for ct in range(KC):
                    for t in range(T):
                        nc.vector.memset(s2t[ct][t][:, MLOC:], 0.0)

                # FC2 in three stages matched to spike-group availability
                # (k=0..3 / 4..7 / 8..11): stage 0 evacuates partial sums
                # (with bias) into y2r on Scalar; later stages accumulate
                # in fresh PSUM and add into y2r on Vector; the last stage
                # also takes BN2 stats from the final y2r on Scalar. No
                # PSUM banks are held across stages, so each stage's 43us
                # of matmul runs dense as soon as its spikes exist.
                KSTAGES = [(0, 4), (4, 8), (8, 12)]
                with tc.tile_pool(name="ps_f", bufs=2, space="PSUM") as ps_f:
                    for si, (ka, kb) in enumerate(KSTAGES):
                        last = si == len(KSTAGES) - 1
                        for ct in range(KC):
                            for ch in range(7):
                                c0, cl = B_CHUNKS[ch]
                                ps2 = ps_f.tile([P, 512], F32,
                                                tag=f"mm2_{si}",
                                                bufs=4 if si == 0 else 2)
                                idx = 0
                                for k in range(ka, kb):
                                    for wsp in (w2h, w2l):
                                        nc.tensor.matmul(
                                            ps2[:, :cl],
                                            wsp[:, k, ct * P:(ct + 1) * P],
                                            s1[k][:, c0:c0 + cl],
                                            start=(idx == 0),
                                            stop=(idx == 2 * (kb - ka) - 1),
                                        )
                                        idx += 1
                                yc = y2r[ct][:, c0:c0 + cl]
                                if si == 0:
                                    nc.scalar.activation(
                                        yc, ps2[:, :cl], AF.Identity,
                                        bias=b2t[:, ct:ct + 1], scale=1.0,
                                    )
                                else:
                                    nc.vector.tensor_tensor(
                                        yc, yc, ps2[:, :cl], OP.add)
                                if last:
                                    id2 = pbs.tile([P, 512], F16, tag="id2")
                                    nc.scalar.activation(
                                        id2[:, :cl], yc, AF.Identity,
                                        bias=0.0, scale=1.0,
                                        accum_out=asum2[:, ct, ch:ch + 1],
                                    )
                                    sq2 = pbs.tile([P, 512], F16, tag="sq2")
                                    nc.scalar.activation(
                                        sq2[:, :cl], yc, AF.Square,
                                        bias=0.0, scale=1.0,
                                        accum_out=asq2[:, ct, ch:ch + 1],
                                    )

                # ---- BN2 stats AllReduce + coeffs --------------------
                st2 = pco2.tile([P, 2 * KC], F32)
                nc.vector.tensor_reduce(st2[:, 0:KC], asum2[:], AX.X, OP.add)
                nc.vector.tensor_reduce(st2[:, KC:2 * KC], asq2[:], AX.X,
                                        OP.add)
                nc.gpsimd.dma_start(st2_in[:], st2[:])
                nc.gpsimd.collective_compute(
                    "AllReduce", OP.add, replica_groups=groups,
                    ins=[st2_in.opt()], outs=[st2_out.opt()],
                )
                stg2 = pco2.tile([P, 2 * KC], F32)
                nc.gpsimd.dma_start(stg2[:], st2_out[:])
                _bn_coeffs(nc, pco2, stg2, g2t[:], be2t[:], dsc2[:, 0:KC],
                           dsh2[:, 0:KC], k15, KC, "bc2")

                # ===== phase C: BN2 affine + LIF2 (vector + gpsimd) ===
                # ===== phase D: PE transpose + fused upcast + store ===
                with (
                    tc.tile_pool(name="pd", bufs=4) as pd,
                    tc.tile_pool(name="pvh2", bufs=2) as pvh2,
                    tc.tile_pool(name="ps_t", bufs=4, space="PSUM") as ps_t,
                ):
                    v2 = [None] * KC

                    def lif2_step(eng, ct, t):
                        d = y2r[ct][:, t * MLOC:(t + 1) * MLOC]
                        # per-partition AP scalars (TensorScalarPtr) are
                        # vector-only; gpsimd handles the immediate-scalar
                        # LIF ops below.
                        nc.vector.tensor_scalar(d, d, dsc2[:, ct:ct + 1],
                                                dsh2[:, ct:ct + 1], OP.mult,
                                                OP.add)
                        st_ = s2t[ct][t]
                        if t == 0:
                            hcur = d
                        else:
                            hh = pvh2.tile([P, MLOC], F32, tag=f"h2_{ct}",
                                           name=f"h2_{ct}_{t}")
                            eng.scalar_tensor_tensor(hh[:], v2[ct][:], 0.5,
                                                     d, OP.mult, OP.add)
                            hcur = hh[:]
                        eng.tensor_scalar(st_[:, 0:MLOC], hcur, 1.0, None,
                                          OP.is_ge)
                        if t < T - 1:
                            vv = pvh2.tile([P, MLOC], F32, tag=f"v2_{ct}",
                                           name=f"v2_{ct}_{t}")
                            eng.scalar_tensor_tensor(vv[:], hcur, 1.0, hcur,
                                                     OP.is_lt, OP.mult)
                            v2[ct] = vv

                    for t in range(T):
                        lif2_step(nc.vector, 0, t)
                        lif2_step(nc.vector, 2, t)
                        lif2_step(nc.vector, 1, t)
                        # transpose/store the 7 m-windows of this t-plane
                        for wi in range(7):
                            m0 = wi * P
                            rl = min(P, MLOC - m0)
                            psT = ps_t.tile([P, 1024], F16, tag="psT")
                            for ct in range(KC):
                                nc.tensor.transpose(
                                    psT[:, ct * P:(ct + 1) * P],
                                    s2t[ct][t][:, m0:m0 + P], identt[:],
                                )
                            ob = pd.tile([P, C], F32, tag="ob")
                            nc.scalar.activation(ob[:rl, :], psT[:rl, 0:C],
                                                 AF.Identity, bias=0.0,
                                                 scale=1.0)
                            r0 = t * MLOC + m0
                            nc.sync.dma_start(out_d[r0:r0 + rl, :],
                                              ob[:rl, :])

    nc.compile()
    return nc


_NC = None
TRACE = False          # set by test harness to capture an NTFF profile
LAST_RESULT = None     # BassKernelResults of the most recent run


def _get_nc():
    global _NC
    if _NC is None:
        _NC = _build()
    return _NC


def _split_f16(a):
    hi = a.astype(np.float16)
    lo = (a - hi.astype(np.float32)).astype(np.float16)
    return np.ascontiguousarray(hi), np.ascontiguousarray(lo)


def _in_maps(x, W1, b1, g1, be1, W2, b2, g2, be2):
    x = np.asarray(x, dtype=np.float32)
    w1t = np.asarray(W1, np.float32).T.reshape(KC, P, H)
    w1thi, w1tlo = _split_f16(w1t)
    w2t = np.asarray(W2, np.float32).T.reshape(KH, P, C)
    w2thi, w2tlo = _split_f16(w2t)
    shared = {
        "w1thi": w1thi, "w1tlo": w1tlo,
        "w2thi": w2thi, "w2tlo": w2tlo,
        "b1": np.asarray(b1, np.float32),
        "g1": np.asarray(g1, np.float32),
        "be1": np.asarray(be1, np.float32),
        "b2": np.asarray(b2, np.float32),
        "g2": np.asarray(g2, np.float32),
        "be2": np.asarray(be2, np.float32),
        "ident": np.eye(P, dtype=np.float16),
    }
    in_maps = []
    for i in range(NCORES):
        xt = x[:, i * BLOC : (i + 1) * BLOC].reshape(R, C).T.reshape(KC, P, R)
        xthi, xtlo = _split_f16(xt)
        in_maps.append({"xthi": xthi, "xtlo": xtlo, **shared})
    return in_maps


def kernel(x, W1, b1, g1, be1, W2, b2, g2, be2):
    nc = _get_nc()
    in_maps = _in_maps(x, W1, b1, g1, be1, W2, b2, g2, be2)
    res = run_bass_kernel_spmd(nc, in_maps, core_ids=list(range(NCORES)),
                               trace=TRACE)
    global LAST_RESULT
    LAST_RESULT = res
    out = np.concatenate(
        [res.results[i]["out"].reshape(T, BLOC, NN, C) for i in range(NCORES)],
        axis=1,
    )
    return out
